# revision 1
# baseline (speedup 1.0000x reference)
"""Trainium2 Bass kernel for a per-token fake-quantized Linear:

    y = fake_quant(fake_quant(x) @ W.T + b)      (per-token int8 symmetric)

x: [4, 2048, 4096] f32, W: [4096, 4096] f32, b: [4096] f32.

Strategy (8 NeuronCores, pure data parallel over tokens - zero collectives):
  - 8192 tokens / 8 cores = 1024 tokens per core; W, b replicated.
  - Per-token quantized x values are integers in [-127, 127], EXACTLY
    representable in bf16, so the matmul runs on TensorE in bf16 (integer
    q as the moving operand, host-pre-packed W.T bf16 stationary) with f32
    PSUM accumulation. The only precision loss vs the f32 reference is W's
    bf16 rounding (~0.8% rel err after output requant; gate is 2e-2).
  - Rounding is exact round-to-nearest-even via +/-1.5*2^23 magic adds.
  - The bias is folded into the matmul as a K=1 rank-1 update
    (b_row^T @ rinv_row) since s_x * rinv_x == 1.
  - z^T = Wb @ q^T is computed in output-transposed layout (W stationary,
    read once per token-half mega-pass); per-token output stats use
    absmax(y) = s_x * absmax(z), computed after an xbar transpose back to
    natural layout via per-o-group DRAM staging.
  - Engine/queue choreography (the hard-won part):
      * q^T for the first token half is built with PE transposes (PE is
        idle during pass 1), the second half via DMA xbar transposes.
      * DMA xbar transposes are issued ONLY on the scalar HWDGE ring -
        transpose-mode and copy-mode running concurrently on the two
        rings corrupts data (shared xbar mode).
      * HWDGE rings are FIFO: data-dependent DMAs (evacuations, rinv)
        head-of-line-block prefetch streams, so they go to the gpsimd
        SWDGE queue or behind-schedule ring slots instead.
      * W is streamed in 1 MiB blocks, alternating rings, re-read for the
        second token-half mega-pass so pass B never waits on input quant.
"""

import sys

if "/opt/trn_rl_repo" not in sys.path:
    sys.path.insert(0, "/opt/trn_rl_repo")

from contextlib import ExitStack

import ml_dtypes
import numpy as np

import concourse.bass as bass
import concourse.mybir as mybir
import concourse.tile as tile
from concourse import bacc
from concourse.bass import ds
from concourse.bass_utils import run_bass_kernel_spmd
from concourse.masks import make_identity

N_CORES = 8
P = 128
T = 1024          # tokens per core
K = 4096          # in features
O = 4096          # out features
TT = T // P       # 8 token tiles
KT = K // P       # 32 k tiles
TH = T // 2       # token half (512) = matmul N
OG = 512          # outputs per o-group (4 o-tiles -> 8 PSUM banks in flight)
NOG = O // OG     # 8 o-groups
OT_PER_G = OG // P  # 4

Q_MAX = 127.0
EPS = 1e-5
MAGIC = 1.5 * 2**23  # f32 add/sub forces round-to-nearest-even to integer
INV_QMAX = float(np.float32(1.0) / np.float32(Q_MAX))

F32 = mybir.dt.float32
BF16 = mybir.dt.bfloat16


def build():
    nc = bacc.Bacc()
    x_ext = nc.declare_dram_parameter("x", [T, K], F32, isOutput=False)
    wt_ext = nc.declare_dram_parameter("wt", [K, O], BF16, isOutput=False)
    b_ext = nc.declare_dram_parameter("b", [O], F32, isOutput=False)
    out_ext = nc.declare_dram_parameter("out", [T, O], F32, isOutput=True)

    with tile.TileContext(nc) as tc, ExitStack() as ctx:
        dram = ctx.enter_context(tc.tile_pool(name="dram", bufs=1, space="DRAM"))
        singles = ctx.enter_context(tc.tile_pool(name="singles", bufs=1))
        xp = ctx.enter_context(tc.tile_pool(name="xp", bufs=5))
        qp = ctx.enter_context(tc.tile_pool(name="qp", bufs=3))
        qt_pool = ctx.enter_context(tc.tile_pool(name="qt", bufs=1))
        sxp = ctx.enter_context(tc.tile_pool(name="sxp", bufs=1))
        stat = ctx.enter_context(tc.tile_pool(name="stat", bufs=3))
        wp = ctx.enter_context(tc.tile_pool(name="wp", bufs=4))
        ztp = ctx.enter_context(tc.tile_pool(name="ztp", bufs=6))
        znp = ctx.enter_context(tc.tile_pool(name="znp", bufs=2))
        yp = ctx.enter_context(tc.tile_pool(name="yp", bufs=2))
        psum = ctx.enter_context(tc.tile_pool(name="psum", bufs=6, space="PSUM"))
        tpp = ctx.enter_context(tc.tile_pool(name="tpp", bufs=2, space="PSUM"))

        zt_dram_og = [dram.tile([OG, T], BF16, tag=f"zt_dram{g}", name=f"zt_dram{g}")
                      for g in range(NOG)]
        q_dram_h = {1: dram.tile([TH, K], BF16, tag="q_dram1", name="q_dram1")}
        rinv_dram = dram.tile([TT, P], F32, tag="rinv_dram")

        identity = singles.tile([P, P], BF16, tag="identity")
        make_identity(nc, identity)

        # bias row in bf16 (partition 0), for the K=1 bias matmul
        b_row = singles.tile([1, O], BF16, tag="b_row")
        nc.gpsimd.dma_start(out=b_row, in_=b_ext[:])  # gpsimd DMA casts f32->bf16

        # q^T strips, one per (token-half, k-tile): [128k, 512t] bf16
        qt_tiles = [
            [qt_pool.tile([P, TH], BF16, tag=f"qt{h}_{k}", name=f"qt{h}_{k}")
             for k in range(KT)]
            for h in range(2)
        ]

        # ---- pass 1: per-token scales + integer quant + q^T transposes ----
        # x is loaded in two 1 MiB half-rows per token tile, split across
        # the two HWDGE rings, so the loads prefetch deeply and neither
        # ring is blocked by a data-dependent DMA.
        KH = K // 2
        sx_tiles = []
        for t in range(TT):
            xh = []
            for i in range(2):
                x_half = xp.tile([P, KH], F32, tag="x_half")
                eng = nc.sync if i == 0 else nc.scalar
                eng.dma_start(
                    out=x_half, in_=x_ext[ds(t * P, P), ds(i * KH, KH)]
                )
                xh.append(x_half)
            amh = stat.tile([P, 2], F32, tag="am_x")
            for i in range(2):
                nc.vector.tensor_reduce(
                    out=amh[:, i:i + 1], in_=xh[i], axis=mybir.AxisListType.X,
                    op=mybir.AluOpType.max, apply_absolute_value=True,
                )
            am = stat.tile([P, 1], F32, tag="am_c")
            nc.vector.tensor_reduce(
                out=am, in_=amh, axis=mybir.AxisListType.X,
                op=mybir.AluOpType.max,
            )
            sx = sxp.tile([P, 1], F32, tag=f"sx{t}", name=f"sx{t}")
            # s = max(absmax, EPS) * (1/127)
            nc.vector.tensor_scalar(
                out=sx, in0=am, scalar1=EPS, scalar2=INV_QMAX,
                op0=mybir.AluOpType.max, op1=mybir.AluOpType.mult,
            )
            rinv = stat.tile([P, 1], F32, tag="rinv_x")
            nc.vector.reciprocal(out=rinv, in_=sx)
            nc.gpsimd.dma_start(out=rinv_dram[t, :], in_=rinv[:, 0:1])
            h, row = t // (TT // 2), (t % (TT // 2)) * P
            for i in range(2):
                # r = x * rinv + MAGIC  (in place, gpsimd), q = r - MAGIC -> bf16
                nc.gpsimd.tensor_scalar(
                    out=xh[i], in0=xh[i], scalar1=rinv, scalar2=MAGIC,
                    op0=mybir.AluOpType.mult, op1=mybir.AluOpType.add,
                )
                q_half = qp.tile([P, KH], BF16, tag="q_half")
                nc.vector.tensor_scalar(
                    out=q_half, in0=xh[i], scalar1=MAGIC,
                    scalar2=None, op0=mybir.AluOpType.subtract,
                )
                if h == 0:
                    # first token half: PE-transpose q into the q^T strips
                    # (PE is idle during pass 1; its program order guarantees
                    # these run before the first matmuls that consume them)
                    for j in range(KT // 2):
                        k = i * (KT // 2) + j
                        tp = tpp.tile([P, P], BF16, tag="tp")
                        nc.tensor.transpose(
                            tp, q_half[:, ds(j * P, P)], identity
                        )
                        nc.scalar.copy(
                            out=qt_tiles[0][k][:, ds(row, P)], in_=tp
                        )
                else:
                    # second half: DRAM-staged xbar transposes (no hurry)
                    nc.gpsimd.dma_start(
                        out=q_dram_h[1][ds(row, P), ds(i * KH, KH)], in_=q_half
                    )
            sx_tiles.append(sx)

        # rinv as a bf16 row vector [1, T] (rhs of the K=1 bias matmul)
        rinv_row = singles.tile([1, T], BF16, tag="rinv_row")
        nc.gpsimd.dma_start(out=rinv_row, in_=rinv_dram[:, :])

        # ---- matmul phase: z^T = Wb @ q^T (+ b * rinv row) ----
        # Two mega-passes over token halves: pass th=0 uses only the first
        # 512 tokens (available early), th=1 re-reads W. W is fetched in
        # 1 MiB blocks of 8 k-subtiles to keep DMAs big.
        KB = 8                       # k-subtiles per W block
        NKB = KT // KB               # 4 blocks per o-group
        for th in range(2):
            for og in range(NOG):
                ps = [
                    psum.tile([P, TH], F32, tag="ps", name=f"ps_{th}_{og}_{i}")
                    for i in range(OT_PER_G)
                ]
                for kb in range(NKB):
                    w_tile = wp.tile([P, KB, OG], BF16, tag="w_tile")
                    if th == 0 and og in (4, 5):
                        # qt-h1 xbar batch owns the scalar ring ~110-160us
                        w_eng = nc.sync
                    else:
                        w_eng = nc.sync if (og * NKB + kb) % 2 == 0 else nc.scalar
                    w_eng.dma_start(
                        out=w_tile,
                        in_=wt_ext[
                            ds(kb * KB * P, KB * P), ds(og * OG, OG)
                        ].rearrange("(s p) o -> p s o", p=P),
                    )
                    for s in range(KB):
                        k = kb * KB + s
                        for ot in range(OT_PER_G):
                            nc.tensor.matmul(
                                ps[ot],
                                w_tile[:, s, ds(ot * P, P)],
                                qt_tiles[th][k],
                                start=(k == 0),
                                stop=False,
                            )
                # bias: psum += b_chunk^T @ rinv_row   (K=1 matmul)
                for ot in range(OT_PER_G):
                    o0 = og * OG + ot * P
                    nc.tensor.matmul(
                        ps[ot],
                        b_row[0:1, ds(o0, P)],
                        rinv_row[0:1, ds(th * TH, TH)],
                        start=False,
                        stop=True,
                    )
                for ot in range(OT_PER_G):
                    zt_sb = ztp.tile([P, TH], BF16, tag="zt_sb")
                    nc.scalar.copy(out=zt_sb, in_=ps[ot])
                    # evacs are data-dependent: on a FIFO HWDGE ring they
                    # head-of-line block later W prefetch, so use SWDGE
                    nc.gpsimd.dma_start(
                        out=zt_dram_og[og][ds(ot * P, P), ds(th * TH, TH)],
                        in_=zt_sb,
                    )
                if th == 0 and og == 3:
                    # second token-half q^T transposes: emitted here so the
                    # scalar ring first serves W-odd for og0-3
                    for k in range(KT):
                        nc.scalar.dma_start_transpose(
                            qt_tiles[1][k], q_dram_h[1][:, ds(k * P, P)]
                        )

        # ---- pass 2: transpose back, scale, requant, store ----
        for t in range(TT):
            z_nat = znp.tile([P, O], BF16, tag="z_nat")
            for g in range(NOG):
                nc.scalar.dma_start_transpose(
                    z_nat[:, ds(g * OG, OG)], zt_dram_og[g][:, ds(t * P, P)]
                )
            # per-token absmax of y comes from z: absmax(y) = s_x * absmax(z)
            am = stat.tile([P, 1], F32, tag="am_z")
            nc.vector.tensor_reduce(
                out=am, in_=z_nat, axis=mybir.AxisListType.X,
                op=mybir.AluOpType.max, apply_absolute_value=True,
            )
            sy = stat.tile([P, 1], F32, tag="sy")
            # sy = (max(am * sx, EPS)) * (1/127)
            nc.vector.tensor_scalar(
                out=sy, in0=am, scalar1=sx_tiles[t], scalar2=EPS,
                op0=mybir.AluOpType.mult, op1=mybir.AluOpType.max,
            )
            nc.vector.tensor_scalar(
                out=sy, in0=sy, scalar1=INV_QMAX, scalar2=None,
                op0=mybir.AluOpType.mult,
            )
            rinvy = stat.tile([P, 1], F32, tag="rinv_y")
            nc.vector.reciprocal(out=rinvy, in_=sy)
            # y = s_x * z (bias already inside z), then round/requant, in
            # half-rows so the f32 y staging fits in SBUF
            OH = O // 2
            for i in range(2):
                y_half = yp.tile([P, OH], F32, tag="y_half")
                nc.scalar.activation(
                    out=y_half, in_=z_nat[:, ds(i * OH, OH)],
                    func=mybir.ActivationFunctionType.Copy, scale=sx_tiles[t],
                )
                # r = y * rinv_y + MAGIC  (in place, gpsimd)
                nc.gpsimd.tensor_scalar(
                    out=y_half, in0=y_half, scalar1=rinvy, scalar2=MAGIC,
                    op0=mybir.AluOpType.mult, op1=mybir.AluOpType.add,
                )
                # y_q = (r - MAGIC) * s_y  (in place)
                nc.vector.tensor_scalar(
                    out=y_half, in0=y_half, scalar1=MAGIC, scalar2=sy,
                    op0=mybir.AluOpType.subtract, op1=mybir.AluOpType.mult,
                )
                nc.sync.dma_start(
                    out=out_ext[ds(t * P, P), ds(i * OH, OH)], in_=y_half
                )

    nc.compile()
    return nc


_NC_CACHE = None


def _get_nc():
    global _NC_CACHE
    if _NC_CACHE is None:
        _NC_CACHE = build()
    return _NC_CACHE


def _run(x, W, b, trace=False):
    nc = _get_nc()
    x2d = np.ascontiguousarray(np.asarray(x, dtype=np.float32).reshape(-1, K))
    wt = np.ascontiguousarray(np.asarray(W, dtype=np.float32).T).astype(
        ml_dtypes.bfloat16
    )
    bf = np.ascontiguousarray(np.asarray(b, dtype=np.float32))
    in_maps = [
        {"x": np.ascontiguousarray(x2d[i * T:(i + 1) * T]), "wt": wt, "b": bf}
        for i in range(N_CORES)
    ]
    res = run_bass_kernel_spmd(nc, in_maps, list(range(N_CORES)), trace=trace)
    out = np.concatenate([res.results[i]["out"] for i in range(N_CORES)], axis=0)
    return out, res


def kernel(x, W, b):
    out, _ = _run(x, W, b, trace=False)
    return out.reshape(np.asarray(x).shape[:-1] + (O,)).astype(np.float32)



# revision 2
# speedup vs baseline: 1.1346x; 1.1346x over previous
"""Trainium2 Bass kernel for a per-token fake-quantized Linear:

    y = fake_quant(fake_quant(x) @ W.T + b)      (per-token int8 symmetric)

x: [4, 2048, 4096] f32, W: [4096, 4096] f32, b: [4096] f32.

Strategy (8 NeuronCores, pure data parallel over tokens - zero collectives):
  - 8192 tokens / 8 cores = 1024 tokens per core; W, b replicated.
  - Per-token quantized x values are integers in [-127, 127], EXACTLY
    representable in bf16, so the matmul runs on TensorE in bf16 with f32
    PSUM accumulation (W pre-packed to bf16 W.T on the host).
  - NATURAL-LAYOUT matmul: stationary operand = q^T tiles [128k, 128t],
    moving operand = W.T [128k, 512o] -> PSUM holds y[token, out] directly.
    No output transposes, no z^T DRAM staging: the per-token output absmax
    is a free-dim reduce folded into PSUM evacuation, and requant runs
    straight out of SBUF at the end of each token-half.
  - Bias: y = s_x*(z + rinv_x*b), applied during PSUM evacuation as ONE
    VectorE scalar_tensor_tensor op (u = b_block*rinv + psum) - no K=1
    bias matmuls on the critical PE stream.
  - Tokens processed in two halves of 512 so half A's requant/output DMA
    overlaps half B's matmuls; W is re-streamed per half (2x33.5 MB still
    well under the DMA roofline).
  - q^T for half A is built with PE transposes (PE is idle during pass 1),
    half B via DRAM-staged xbar transposes on the scalar HWDGE ring,
    overlapped with half A's matmuls.
  - Data-dependent DMAs (x half-B loads, q staging, output stores) ride
    the gpsimd SWDGE queue so they never head-of-line-block the W
    prefetch streams on the two HWDGE rings.
  - Rounding is exact round-to-nearest-even via +/-1.5*2^23 magic adds.
"""

import sys

if "/opt/trn_rl_repo" not in sys.path:
    sys.path.insert(0, "/opt/trn_rl_repo")

from contextlib import ExitStack

import ml_dtypes
import numpy as np

import concourse.bass as bass
import concourse.mybir as mybir
import concourse.tile as tile
from concourse import bacc
from concourse.bass import ds
from concourse.bass_utils import run_bass_kernel_spmd
from concourse.masks import make_identity

N_CORES = 8
P = 128
T = 1024          # tokens per core
K = 4096          # in features
O = 4096          # out features
TT = T // P       # 8 token tiles
KT = K // P       # 32 k tiles
TH = T // 2       # tokens per half (512)
TTH = TT // 2     # token tiles per half (4)
OC = 512          # outputs per o-chunk (one PSUM bank of f32)
NOC = O // OC     # 8 o-chunks
KB = 8            # k-subtiles per W block (1 MiB)
NKB = KT // KB    # 4 W blocks per o-chunk

Q_MAX = 127.0
EPS = 1e-5
MAGIC = 1.5 * 2**23  # f32 add/sub forces round-to-nearest-even to integer
INV_QMAX = float(np.float32(1.0) / np.float32(Q_MAX))

F32 = mybir.dt.float32
BF16 = mybir.dt.bfloat16


def build():
    nc = bacc.Bacc()
    x_ext = nc.declare_dram_parameter("x", [T, K], F32, isOutput=False)
    wt_ext = nc.declare_dram_parameter("wt", [K, O], BF16, isOutput=False)
    b_ext = nc.declare_dram_parameter("b", [O], F32, isOutput=False)
    out_ext = nc.declare_dram_parameter("out", [T, O], F32, isOutput=True)

    with tile.TileContext(nc) as tc, ExitStack() as ctx:
        dram = ctx.enter_context(tc.tile_pool(name="dram", bufs=1, space="DRAM"))
        singles = ctx.enter_context(tc.tile_pool(name="singles", bufs=1))
        xp = ctx.enter_context(tc.tile_pool(name="xp", bufs=5))
        qp = ctx.enter_context(tc.tile_pool(name="qp", bufs=3))
        qt_pool = ctx.enter_context(tc.tile_pool(name="qt", bufs=1))
        sxp = ctx.enter_context(tc.tile_pool(name="sxp", bufs=1))
        stat = ctx.enter_context(tc.tile_pool(name="stat", bufs=4))
        wp = ctx.enter_context(tc.tile_pool(name="wp", bufs=3))
        up = ctx.enter_context(tc.tile_pool(name="up", bufs=1))
        yp = ctx.enter_context(tc.tile_pool(name="yp", bufs=2))
        psum = ctx.enter_context(tc.tile_pool(name="psum", bufs=6, space="PSUM"))
        tpp = ctx.enter_context(tc.tile_pool(name="tpp", bufs=2, space="PSUM"))

        # DRAM staging for half-B q (xbar-transposed back later)
        q_dram = dram.tile([TH, K], BF16, tag="q_dram", name="q_dram")

        identity = singles.tile([P, P], BF16, tag="identity")
        make_identity(nc, identity)

        # bias replicated across partitions: [128, O] bf16
        b_row = singles.tile([1, O], BF16, tag="b_row")
        nc.gpsimd.dma_start(out=b_row, in_=b_ext[:])  # gpsimd DMA casts f32->bf16
        b_block = singles.tile([P, O], BF16, tag="b_block")
        nc.gpsimd.partition_broadcast(b_block, b_row)

        # q^T strips, one per k-tile: [128k, 1024t] bf16
        qt_tiles = [
            qt_pool.tile([P, T], BF16, tag=f"qt{k}", name=f"qt{k}")
            for k in range(KT)
        ]
        # per-token input scale / inverse scale, named per t-tile
        sx_tiles = [sxp.tile([P, 1], F32, tag=f"sx{t}", name=f"sx{t}")
                    for t in range(TT)]
        rinv_tiles = [sxp.tile([P, 1], F32, tag=f"rinv{t}", name=f"rinv{t}")
                      for t in range(TT)]
        # per-(t-tile, o-chunk) partial |u| maxima
        amp_tiles = [sxp.tile([P, NOC], F32, tag=f"amp{t}", name=f"amp{t}")
                     for t in range(TT)]
        # u = z + rinv_x*b staged per t-tile of the current half, bf16
        u_tiles = [up.tile([P, O], BF16, tag=f"u{i}", name=f"u{i}")
                   for i in range(TTH)]

        KH = K // 2

        # ---- pass 1: per-token scales + integer quant + q^T build ----
        def pass1(t):
            h = t // TTH
            xh = []
            for i in range(2):
                x_half = xp.tile([P, KH], F32, tag="x_half")
                if h == 0:
                    eng = nc.sync if i == 0 else nc.scalar
                else:
                    # half B rides SWDGE: keeps the HWDGE rings free for W
                    eng = nc.gpsimd
                eng.dma_start(out=x_half, in_=x_ext[ds(t * P, P), ds(i * KH, KH)])
                xh.append(x_half)
            amh = stat.tile([P, 2], F32, tag="am_x")
            for i in range(2):
                nc.vector.tensor_reduce(
                    out=amh[:, i:i + 1], in_=xh[i], axis=mybir.AxisListType.X,
                    op=mybir.AluOpType.max, apply_absolute_value=True,
                )
            am = stat.tile([P, 1], F32, tag="am_c")
            nc.vector.tensor_reduce(
                out=am, in_=amh, axis=mybir.AxisListType.X, op=mybir.AluOpType.max,
            )
            # s = max(absmax, EPS) * (1/127)
            nc.vector.tensor_scalar(
                out=sx_tiles[t], in0=am, scalar1=EPS, scalar2=INV_QMAX,
                op0=mybir.AluOpType.max, op1=mybir.AluOpType.mult,
            )
            nc.vector.reciprocal(out=rinv_tiles[t], in_=sx_tiles[t])
            for i in range(2):
                # r = x * rinv + MAGIC  (in place, gpsimd), q = r - MAGIC -> bf16
                nc.gpsimd.tensor_scalar(
                    out=xh[i], in0=xh[i], scalar1=rinv_tiles[t], scalar2=MAGIC,
                    op0=mybir.AluOpType.mult, op1=mybir.AluOpType.add,
                )
                q_half = qp.tile([P, KH], BF16, tag="q_half")
                nc.vector.tensor_scalar(
                    out=q_half, in0=xh[i], scalar1=MAGIC,
                    scalar2=None, op0=mybir.AluOpType.subtract,
                )
                if h == 0:
                    # half A: PE-transpose q into the q^T strips (PE idle now)
                    for j in range(KT // 2):
                        k = i * (KT // 2) + j
                        tp = tpp.tile([P, P], BF16, tag="tp")
                        nc.tensor.transpose(tp, q_half[:, ds(j * P, P)], identity)
                        nc.scalar.copy(out=qt_tiles[k][:, ds(t * P, P)], in_=tp)
                else:
                    # half B: DRAM-stage; xbar transposes issued later
                    nc.gpsimd.dma_start(
                        out=q_dram[ds((t - TTH) * P, P), ds(i * KH, KH)],
                        in_=q_half,
                    )

        for t in range(TT):
            pass1(t)

        # ---- matmul + fused evac/requant, one token-half at a time ----
        def matmul_half(h):
            t0 = h * TTH
            for oc in range(NOC):
                ps = [psum.tile([P, OC], F32, tag="ps", name=f"ps_{h}_{oc}_{i}")
                      for i in range(TTH)]
                for kb in range(NKB):
                    w_tile = wp.tile([P, KB, OC], BF16, tag="w_tile")
                    w_eng = nc.sync if (oc * NKB + kb) % 2 == 0 else nc.scalar
                    w_eng.dma_start(
                        out=w_tile,
                        in_=wt_ext[
                            ds(kb * KB * P, KB * P), ds(oc * OC, OC)
                        ].rearrange("(s p) o -> p s o", p=P),
                    )
                    for s in range(KB):
                        k = kb * KB + s
                        for i in range(TTH):
                            nc.tensor.matmul(
                                ps[i],
                                qt_tiles[k][:, ds((t0 + i) * P, P)],
                                w_tile[:, s, :],
                                start=(k == 0),
                                stop=(k == KT - 1),
                            )
                for i in range(TTH):
                    t = t0 + i
                    # u = b*rinv_x + z   (one VectorE op, PSUM -> SBUF bf16)
                    nc.vector.scalar_tensor_tensor(
                        out=u_tiles[i][:, ds(oc * OC, OC)],
                        in0=b_block[:, ds(oc * OC, OC)],
                        scalar=rinv_tiles[t],
                        in1=ps[i],
                        op0=mybir.AluOpType.mult,
                        op1=mybir.AluOpType.add,
                    )
                    nc.vector.tensor_reduce(
                        out=amp_tiles[t][:, oc:oc + 1],
                        in_=u_tiles[i][:, ds(oc * OC, OC)],
                        axis=mybir.AxisListType.X,
                        op=mybir.AluOpType.max, apply_absolute_value=True,
                    )
            if h == 0:
                # half B q^T xbar transposes: scalar ring, after half A's W
                for k in range(KT):
                    nc.scalar.dma_start_transpose(
                        qt_tiles[k][:, ds(TH, TH)], q_dram[:, ds(k * P, P)]
                    )

        def requant_half(h):
            t0 = h * TTH
            OH = O // 2
            for i in range(TTH):
                t = t0 + i
                am = stat.tile([P, 1], F32, tag="am_u")
                nc.vector.tensor_reduce(
                    out=am, in_=amp_tiles[t], axis=mybir.AxisListType.X,
                    op=mybir.AluOpType.max,
                )
                # s_y = max(s_x * absmax_u, EPS) * (1/127)
                sy = stat.tile([P, 1], F32, tag="sy")
                nc.vector.tensor_scalar(
                    out=sy, in0=am, scalar1=sx_tiles[t], scalar2=EPS,
                    op0=mybir.AluOpType.mult, op1=mybir.AluOpType.max,
                )
                nc.vector.tensor_scalar(
                    out=sy, in0=sy, scalar1=INV_QMAX, scalar2=None,
                    op0=mybir.AluOpType.mult,
                )
                rinvy = stat.tile([P, 1], F32, tag="rinv_y")
                nc.vector.reciprocal(out=rinvy, in_=sy)
                # f = s_x * rinv_y : y*rinv_y == u*f
                f = stat.tile([P, 1], F32, tag="f")
                nc.vector.tensor_scalar(
                    out=f, in0=rinvy, scalar1=sx_tiles[t], scalar2=None,
                    op0=mybir.AluOpType.mult,
                )
                for c in range(2):
                    y_half = yp.tile([P, OH], F32, tag="y_half")
                    # r = u*f + MAGIC (gpsimd), y_q = (r - MAGIC)*s_y (vector)
                    nc.gpsimd.tensor_scalar(
                        out=y_half, in0=u_tiles[i][:, ds(c * OH, OH)],
                        scalar1=f, scalar2=MAGIC,
                        op0=mybir.AluOpType.mult, op1=mybir.AluOpType.add,
                    )
                    nc.vector.tensor_scalar(
                        out=y_half, in0=y_half, scalar1=MAGIC, scalar2=sy,
                        op0=mybir.AluOpType.subtract, op1=mybir.AluOpType.mult,
                    )
                    nc.gpsimd.dma_start(
                        out=out_ext[ds(t * P, P), ds(c * OH, OH)], in_=y_half
                    )

        for h in range(2):
            matmul_half(h)
            requant_half(h)

    nc.compile()
    return nc


_NC_CACHE = None


def _get_nc():
    global _NC_CACHE
    if _NC_CACHE is None:
        _NC_CACHE = build()
    return _NC_CACHE


def _run(x, W, b, trace=False):
    nc = _get_nc()
    x2d = np.ascontiguousarray(np.asarray(x, dtype=np.float32).reshape(-1, K))
    wt = np.ascontiguousarray(np.asarray(W, dtype=np.float32).T).astype(
        ml_dtypes.bfloat16
    )
    bf = np.ascontiguousarray(np.asarray(b, dtype=np.float32))
    in_maps = [
        {"x": np.ascontiguousarray(x2d[i * T:(i + 1) * T]), "wt": wt, "b": bf}
        for i in range(N_CORES)
    ]
    res = run_bass_kernel_spmd(nc, in_maps, list(range(N_CORES)), trace=trace)
    out = np.concatenate([res.results[i]["out"] for i in range(N_CORES)], axis=0)
    return out, res


def kernel(x, W, b):
    out, _ = _run(x, W, b, trace=False)
    return out.reshape(np.asarray(x).shape[:-1] + (O,)).astype(np.float32)
